# revision 8
# baseline (speedup 1.0000x reference)
"""KroneckerLSTM trn2 kernel.

Computes, for 8 gate-klins (L @ t @ R + b, t in {x,h}):
    i = sigmoid(klin_ii(x) + klin_hi(h)); f = sigmoid(...); g = tanh(...); o = sigmoid(...)
    c_new = f*c + i*g ; h_new = o*tanh(c_new)
Returns (h_new, c_new), each [1024,1024] f32.

Sharding: output rows split across 8 cores (128 rows each) -> zero collectives.
Per core, for each gate g:  B_g[rows,:] = (L_g[rows,:] @ t) @ R_g
  mm1 computes A^T directly (lhsT = t tiles (natural), rhs = host-pretransposed
  L^T column-slices, stacked 4 gates wide so N=512), so mm1's PSUM output is the
  lhsT for mm2 (rhs = R_g in natural layout).  The x-klin and h-klin of each
  gate pair accumulate into the same PSUM bank; bias is added in-place in PSUM.

All DMA'd tensors are bf16 (host-side cast): per-core HBM traffic is ~24MB.
DMA-instruction issue is descriptor-rate-limited (~5ns per partition line), so
all big tensors are host-packed into [128, K*freedim] layouts whose partition
lines are 8KB-contiguous in DRAM; each DMA instruction then moves 0.5-2MB with
only 128 descriptor lines.  Input loads stream on the SP HWDGE ring in
consumption order; output stores use the Activation ring so they never block
input streaming.
"""

import sys

import numpy as np

if "/opt/trn_rl_repo" not in sys.path:
    sys.path.insert(0, "/opt/trn_rl_repo")

N = 1024
M = 1024
P = 128
NC = 8
KT = N // P  # 8 k-tiles of 128
# gate pairs in order i, f, g, o: (x-gate, h-gate, activation)
PAIRS = [("ii", "hi", "Sigmoid"), ("if", "hf", "Sigmoid"),
         ("ig", "hg", "Tanh"), ("io", "ho", "Sigmoid")]

_cache = {}


def _build_program():
    import concourse.bass as bass
    import concourse.mybir as mybir
    import concourse.tile as tile
    from concourse import bacc
    from concourse.bass import ts

    FP = mybir.dt.float32
    BF = mybir.dt.bfloat16
    AF = mybir.ActivationFunctionType

    nc = bacc.Bacc("TRN2", target_bir_lowering=False, debug=False,
                   enable_asserts=False, num_devices=NC)

    # packed layouts: [128, K*freedim], partition p / chunk k = source row k*128+p
    xp_d = nc.dram_tensor("xp", [P, KT * M], BF, kind="ExternalInput").ap()
    hp_d = nc.dram_tensor("hp", [P, KT * M], BF, kind="ExternalInput").ap()
    ltx_d = nc.dram_tensor("ltxp", [P, KT * 512], BF, kind="ExternalInput").ap()
    lth_d = nc.dram_tensor("lthp", [P, KT * 512], BF, kind="ExternalInput").ap()
    rx_d = [nc.dram_tensor(f"rxp{p}", [P, KT * M], BF, kind="ExternalInput").ap()
            for p in range(4)]
    rh_d = [nc.dram_tensor(f"rhp{p}", [P, KT * M], BF, kind="ExternalInput").ap()
            for p in range(4)]
    bs_d = nc.dram_tensor("bsp", [P, 4 * M], BF, kind="ExternalInput").ap()
    c_d = nc.dram_tensor("cprev", [P, M], BF, kind="ExternalInput").ap()
    hn_d = nc.dram_tensor("h_new", [P, M], BF, kind="ExternalOutput").ap()
    cn_d = nc.dram_tensor("c_new", [P, M], BF, kind="ExternalOutput").ap()

    with tile.TileContext(nc) as tc:
        from contextlib import ExitStack
        with ExitStack() as ctx:
            big = ctx.enter_context(tc.tile_pool(name="big", bufs=1))
            atp = ctx.enter_context(tc.tile_pool(name="at", bufs=1))
            rp = ctx.enter_context(tc.tile_pool(name="rstream", bufs=6))
            psp = ctx.enter_context(tc.tile_pool(name="ps", bufs=8, space="PSUM"))
            gp = ctx.enter_context(tc.tile_pool(name="gates", bufs=1))
            ew = ctx.enter_context(tc.tile_pool(name="ew", bufs=1))
            wp = ctx.enter_context(tc.tile_pool(name="warm", bufs=1))

            # small PE warm-up burst overlapping the DMA prologue
            wa = wp.tile([P, P], BF, tag="wa")
            wb = wp.tile([P, 512], BF, tag="wb")
            nc.vector.memset(wa[:], 0.0)
            nc.vector.memset(wb[:], 0.0)
            wps = psp.tile([P, 512], FP, tag="bank", name="warm_ps")
            for w in range(6):
                nc.tensor.matmul(wps[:], wa[:], wb[:], start=True, stop=True,
                                 skip_group_check=True)

            # ---- all input loads, issued upfront in consumption order (SP
            # ring, FIFO).  x/h stream per k-tile so mm1 starts ~1us after the
            # first bytes land.
            ltxt = big.tile([P, KT * 512], BF, tag="ltx")
            nc.sync.dma_start(ltxt[:, 0:512], ltx_d[:, 0:512])
            xcs = [big.tile([P, M], BF, tag=f"xc{b}", name=f"xc{b}")
                   for b in range(KT)]
            nc.sync.dma_start(xcs[0][:], xp_d[:, 0:M])
            nc.sync.dma_start(ltxt[:, 512:KT * 512], ltx_d[:, 512:KT * 512])
            for b in range(1, KT):
                nc.sync.dma_start(xcs[b][:], xp_d[:, ts(b, M)])
            ltht = big.tile([P, KT * 512], BF, tag="lth")
            nc.sync.dma_start(ltht[:], lth_d[:])
            hcs = [big.tile([P, M], BF, tag=f"hc{b}", name=f"hc{b}")
                   for b in range(KT)]
            for b in range(KT):
                nc.sync.dma_start(hcs[b][:], hp_d[:, ts(b, M)])
            bst = big.tile([P, 4 * M], BF, tag="bs")
            nc.sync.dma_start(bst[:], bs_d[:])
            cs = ew.tile([P, M], BF, tag="cs")
            nc.sync.dma_start(cs[:], c_d[:])

            # mm1: at_s[j][mloc, 4*128] = sum_k t[k, j*128+mloc] * LT[k, col]
            ats = {"x": [], "h": []}
            for s, tch, ltt in (("x", xcs, ltxt), ("h", hcs, ltht)):
                pts = [psp.tile([P, 4 * P], FP, tag="bank", name=f"pt_{s}_{j}")
                       for j in range(KT)]
                for kc in range(KT):
                    tt = tch[kc]
                    lts = ltt[:, ts(kc, 512)]
                    for j in range(KT):
                        nc.tensor.matmul(pts[j][:], tt[:, ts(j, P)], lts,
                                         start=(kc == 0), stop=(kc == KT - 1))
                for j in range(KT):
                    at = atp.tile([P, 4 * P], BF, tag=f"at{s}{j}")
                    nc.vector.tensor_copy(at[:], pts[j][:])
                    ats[s].append(at)

            # mm2 per gate pair (+ bias + activation)
            gates = []

            def pair_mm(p, actname):
                pt0 = psp.tile([P, 512], FP, tag="bank", name=f"p{p}b0")
                pt1 = psp.tile([P, 512], FP, tag="bank", name=f"p{p}b1")
                for s, rd in (("x", rx_d[p]), ("h", rh_d[p])):
                    for b in range(2):  # half-matrix chunks of 4 k-tiles
                        rt = rp.tile([P, 4 * M], BF, tag="r")
                        nc.sync.dma_start(rt[:], rd[:, ts(b, 4 * M)])
                        for jj in range(4):
                            j = 4 * b + jj
                            first = (s == "x") and (j == 0)
                            last = (s == "h") and (j == KT - 1)
                            lhsT = ats[s][j][:, ts(p, P)]
                            nc.tensor.matmul(pt0[:], lhsT,
                                             rt[:, jj * M: jj * M + 512],
                                             start=first, stop=last)
                            nc.tensor.matmul(pt1[:], lhsT,
                                             rt[:, jj * M + 512: (jj + 1) * M],
                                             start=first, stop=last)
                gt = gp.tile([P, M], BF, tag=f"g{p}")
                af = getattr(AF, actname)
                halves = 2 if p == 3 else 1  # fine-grain the o-gate tail
                for bb in range(2):
                    pt = (pt0, pt1)[bb]
                    for q in range(halves):
                        w = 512 // halves
                        nc.vector.tensor_add(pt[:, ts(q, w)], pt[:, ts(q, w)],
                                             bst[:, p * M + bb * 512 + q * w:
                                                 p * M + bb * 512 + (q + 1) * w])
                        nc.scalar.activation(
                            gt[:, bb * 512 + q * w: bb * 512 + (q + 1) * w],
                            pt[:, ts(q, w)], af)
                gates.append(gt)

            for p in range(3):  # i, f, g
                pair_mm(p, PAIRS[p][2])
            gi, gf, gg = gates

            # c_new chain overlaps the o-gate matmuls; stores go on the
            # Activation HWDGE ring to keep the SP ring free for R streaming
            fc = ew.tile([P, M], FP, tag="fc")
            ig = ew.tile([P, M], FP, tag="ig")
            cn = ew.tile([P, M], BF, tag="cn")
            tch_t = ew.tile([P, M], BF, tag="tch")
            for hf in range(2):
                sl = ts(hf, 512)
                nc.vector.tensor_mul(fc[:, sl], gf[:, sl], cs[:, sl])
                nc.vector.tensor_mul(ig[:, sl], gi[:, sl], gg[:, sl])
                nc.vector.tensor_add(cn[:, sl], fc[:, sl], ig[:, sl])
                nc.scalar.dma_start(cn_d[:, sl], cn[:, sl])
                nc.scalar.activation(tch_t[:, sl], cn[:, sl], AF.Tanh)

            pair_mm(3, PAIRS[3][2])  # o
            go = gates[3]
            hn = ew.tile([P, M], BF, tag="hn")
            for qf in range(4):  # quarters: shortens the post-last-matmul tail
                sl = ts(qf, 256)
                nc.vector.tensor_mul(hn[:, sl], go[:, sl], tch_t[:, sl])
                nc.scalar.dma_start(hn_d[:, sl], hn[:, sl])

    nc.compile()
    return nc


def _get_program():
    if "nc" not in _cache:
        _cache["nc"] = _build_program()
    return _cache["nc"]


def _pack(a):
    # [R*128, C] -> [128, R*C]; out[p, k*C+c] = a[k*128+p, c]
    r = a.shape[0] // P
    return np.ascontiguousarray(
        a.reshape(r, P, a.shape[1]).transpose(1, 0, 2).reshape(P, r * a.shape[1]))


def _prep_in_maps(inputs):
    import ml_dtypes
    BF = ml_dtypes.bfloat16
    bf = lambda a: np.asarray(a, dtype=np.float32).astype(BF)
    xp = _pack(bf(inputs["x"]))
    hp = _pack(bf(inputs["h"]))
    c = np.asarray(inputs["c"], dtype=np.float32)
    LTx = [bf(np.asarray(inputs[f"L_{xg}"]).T) for xg, _, _ in PAIRS]
    LTh = [bf(np.asarray(inputs[f"L_{hg}"]).T) for _, hg, _ in PAIRS]
    Rxp = [_pack(bf(inputs[f"R_{xg}"])) for xg, _, _ in PAIRS]
    Rhp = [_pack(bf(inputs[f"R_{hg}"])) for _, hg, _ in PAIRS]
    bsum = [(np.asarray(inputs[f"b_{xg}"], dtype=np.float32)
             + np.asarray(inputs[f"b_{hg}"], dtype=np.float32)).astype(BF)
            for xg, hg, _ in PAIRS]

    in_maps = []
    for k in range(NC):
        sl = slice(P * k, P * (k + 1))
        im = {
            "xp": xp, "hp": hp,
            "ltxp": _pack(np.ascontiguousarray(
                np.concatenate([lt[:, sl] for lt in LTx], axis=1))),
            "lthp": _pack(np.ascontiguousarray(
                np.concatenate([lt[:, sl] for lt in LTh], axis=1))),
            "bsp": _pack(np.ascontiguousarray(
                np.concatenate([b[sl] for b in bsum], axis=0))),
            "cprev": np.ascontiguousarray(c[sl].astype(BF)),
        }
        for p in range(4):
            im[f"rxp{p}"] = Rxp[p]
            im[f"rhp{p}"] = Rhp[p]
        in_maps.append(im)
    return in_maps


def kernel(**inputs):
    from concourse.bass_utils import run_bass_kernel_spmd

    nc = _get_program()
    in_maps = _prep_in_maps(inputs)
    res = run_bass_kernel_spmd(nc, in_maps, core_ids=list(range(NC)))
    h_new = np.concatenate(
        [np.asarray(res.results[k]["h_new"], dtype=np.float32) for k in range(NC)],
        axis=0)
    c_new = np.concatenate(
        [np.asarray(res.results[k]["c_new"], dtype=np.float32) for k in range(NC)],
        axis=0)
    return (h_new, c_new)


# revision 9
# speedup vs baseline: 1.0022x; 1.0022x over previous
"""KroneckerLSTM trn2 kernel.

Computes, for 8 gate-klins (L @ t @ R + b, t in {x,h}):
    i = sigmoid(klin_ii(x) + klin_hi(h)); f = sigmoid(...); g = tanh(...); o = sigmoid(...)
    c_new = f*c + i*g ; h_new = o*tanh(c_new)
Returns (h_new, c_new), each [1024,1024] f32.

Sharding: output rows split across 8 cores (128 rows each) -> zero collectives.
Per core, for each gate g:  B_g[rows,:] = (L_g[rows,:] @ t) @ R_g
  mm1 computes A^T directly (lhsT = t tiles (natural), rhs = host-pretransposed
  L^T column-slices, stacked 4 gates wide so N=512), so mm1's PSUM output is the
  lhsT for mm2 (rhs = R_g in natural layout).  The x-klin and h-klin of each
  gate pair accumulate into the same PSUM bank; bias is added in-place in PSUM.

All DMA'd tensors are bf16 (host-side cast): per-core HBM traffic is ~24MB.
DMA-instruction issue is descriptor-rate-limited (~5ns per partition line), so
all big tensors are host-packed into [128, K*freedim] layouts whose partition
lines are 8KB-contiguous in DRAM; each DMA instruction then moves 0.5-2MB with
only 128 descriptor lines.  Input loads stream on the SP HWDGE ring in
consumption order; output stores use the Activation ring so they never block
input streaming.
"""

import sys

import numpy as np

if "/opt/trn_rl_repo" not in sys.path:
    sys.path.insert(0, "/opt/trn_rl_repo")

N = 1024
M = 1024
P = 128
NC = 8
KT = N // P  # 8 k-tiles of 128
# gate pairs in order i, f, g, o: (x-gate, h-gate, activation)
PAIRS = [("ii", "hi", "Sigmoid"), ("if", "hf", "Sigmoid"),
         ("ig", "hg", "Tanh"), ("io", "ho", "Sigmoid")]

_cache = {}


def _build_program():
    import concourse.bass as bass
    import concourse.mybir as mybir
    import concourse.tile as tile
    from concourse import bacc
    from concourse.bass import ts

    FP = mybir.dt.float32
    BF = mybir.dt.bfloat16
    AF = mybir.ActivationFunctionType

    nc = bacc.Bacc("TRN2", target_bir_lowering=False, debug=False,
                   enable_asserts=False, num_devices=NC)

    # packed layouts: [128, K*freedim], partition p / chunk k = source row k*128+p
    xp_d = nc.dram_tensor("xp", [P, KT * M], BF, kind="ExternalInput").ap()
    hp_d = nc.dram_tensor("hp", [P, KT * M], BF, kind="ExternalInput").ap()
    ltx_d = nc.dram_tensor("ltxp", [P, KT * 512], BF, kind="ExternalInput").ap()
    lth_d = nc.dram_tensor("lthp", [P, KT * 512], BF, kind="ExternalInput").ap()
    rx_d = [nc.dram_tensor(f"rxp{p}", [P, KT * M], BF, kind="ExternalInput").ap()
            for p in range(4)]
    rh_d = [nc.dram_tensor(f"rhp{p}", [P, KT * M], BF, kind="ExternalInput").ap()
            for p in range(4)]
    bs_d = nc.dram_tensor("bsp", [P, 4 * M], BF, kind="ExternalInput").ap()
    c_d = nc.dram_tensor("cprev", [P, M], BF, kind="ExternalInput").ap()
    hn_d = nc.dram_tensor("h_new", [P, M], BF, kind="ExternalOutput").ap()
    cn_d = nc.dram_tensor("c_new", [P, M], BF, kind="ExternalOutput").ap()

    with tile.TileContext(nc) as tc:
        from contextlib import ExitStack
        with ExitStack() as ctx:
            big = ctx.enter_context(tc.tile_pool(name="big", bufs=1))
            atp = ctx.enter_context(tc.tile_pool(name="at", bufs=1))
            rp = ctx.enter_context(tc.tile_pool(name="rstream", bufs=6))
            psp = ctx.enter_context(tc.tile_pool(name="ps", bufs=8, space="PSUM"))
            gp = ctx.enter_context(tc.tile_pool(name="gates", bufs=1))
            ew = ctx.enter_context(tc.tile_pool(name="ew", bufs=1))
            wp = ctx.enter_context(tc.tile_pool(name="warm", bufs=1))

            # small PE warm-up burst overlapping the DMA prologue
            wa = wp.tile([P, P], BF, tag="wa")
            wb = wp.tile([P, 512], BF, tag="wb")
            nc.vector.memset(wa[:], 0.0)
            nc.vector.memset(wb[:], 0.0)
            wps = psp.tile([P, 512], FP, tag="bank", name="warm_ps")
            for w in range(6):
                nc.tensor.matmul(wps[:], wa[:], wb[:], start=True, stop=True,
                                 skip_group_check=True)

            # ---- all input loads, issued upfront in consumption order (SP
            # ring, FIFO).  x/h stream per k-tile so mm1 starts ~1us after the
            # first bytes land.
            ltxt = big.tile([P, KT * 512], BF, tag="ltx")
            nc.sync.dma_start(ltxt[:, 0:512], ltx_d[:, 0:512])
            xcs = [big.tile([P, M], BF, tag=f"xc{b}", name=f"xc{b}")
                   for b in range(KT)]
            nc.sync.dma_start(xcs[0][:], xp_d[:, 0:M])
            nc.sync.dma_start(ltxt[:, 512:KT * 512], ltx_d[:, 512:KT * 512])
            for b in range(1, KT):
                nc.sync.dma_start(xcs[b][:], xp_d[:, ts(b, M)])
            ltht = big.tile([P, KT * 512], BF, tag="lth")
            nc.sync.dma_start(ltht[:], lth_d[:])
            hcs = [big.tile([P, M], BF, tag=f"hc{b}", name=f"hc{b}")
                   for b in range(KT)]
            for b in range(KT):
                nc.sync.dma_start(hcs[b][:], hp_d[:, ts(b, M)])
            bst = big.tile([P, 4 * M], BF, tag="bs")
            nc.sync.dma_start(bst[:], bs_d[:])
            cs = ew.tile([P, M], BF, tag="cs")
            nc.sync.dma_start(cs[:], c_d[:])

            # mm1-x: at_x[j][mloc, 4*128] = sum_k x[k, j*128+mloc] * LTx[k, col]
            ats = {"x": [None] * KT, "h": [None] * KT}
            pts = [psp.tile([P, 4 * P], FP, tag="bank", name=f"pt_x_{j}")
                   for j in range(KT)]
            for kc in range(KT):
                tt = xcs[kc]
                lts = ltxt[:, ts(kc, 512)]
                for j in range(KT):
                    nc.tensor.matmul(pts[j][:], tt[:, ts(j, P)], lts,
                                     start=(kc == 0), stop=(kc == KT - 1))
            for j in range(KT):
                at = atp.tile([P, 4 * P], BF, tag=f"atx{j}", name=f"atx{j}")
                nc.vector.tensor_copy(at[:], pts[j][:])
                ats["x"][j] = at

            # SBUF fp32 partials for the mm2 x-passes (bias folded in)
            pxp = ctx.enter_context(tc.tile_pool(name="px", bufs=1))
            pxs = [pxp.tile([P, M], FP, tag=f"px{p}", name=f"px{p}")
                   for p in range(4)]
            gates = [gp.tile([P, M], BF, tag=f"g{p}", name=f"g{p}")
                     for p in range(4)]

            def mm1h_wave(w):
                # mm1-h for j in {2w, 2w+1}: fills PE time while R streams
                pw = [psp.tile([P, 4 * P], FP, tag="bank", name=f"pt_h_{2*w+i}")
                      for i in range(2)]
                for kc in range(KT):
                    lts = ltht[:, ts(kc, 512)]
                    for i in range(2):
                        j = 2 * w + i
                        nc.tensor.matmul(pw[i][:], hcs[kc][:, ts(j, P)], lts,
                                         start=(kc == 0), stop=(kc == KT - 1))
                for i in range(2):
                    j = 2 * w + i
                    at = atp.tile([P, 4 * P], BF, tag=f"ath{j}", name=f"ath{j}")
                    nc.vector.tensor_copy(at[:], pw[i][:])
                    ats["h"][j] = at

            def pair_x(p):
                pt0 = psp.tile([P, 512], FP, tag="bank", name=f"p{p}xb0")
                pt1 = psp.tile([P, 512], FP, tag="bank", name=f"p{p}xb1")
                for b in range(2):  # half-matrix chunks of 4 k-tiles
                    rt = rp.tile([P, 4 * M], BF, tag="r")
                    nc.sync.dma_start(rt[:], rx_d[p][:, ts(b, 4 * M)])
                    for jj in range(4):
                        j = 4 * b + jj
                        lhsT = ats["x"][j][:, ts(p, P)]
                        nc.tensor.matmul(pt0[:], lhsT,
                                         rt[:, jj * M: jj * M + 512],
                                         start=(j == 0), stop=(j == KT - 1))
                        nc.tensor.matmul(pt1[:], lhsT,
                                         rt[:, jj * M + 512: (jj + 1) * M],
                                         start=(j == 0), stop=(j == KT - 1))
                # copy out of PSUM with the pair bias folded in (frees banks)
                nc.vector.tensor_add(pxs[p][:, 0:512], pt0[:],
                                     bst[:, p * M: p * M + 512])
                nc.vector.tensor_add(pxs[p][:, 512:M], pt1[:],
                                     bst[:, p * M + 512: (p + 1) * M])

            def pair_h(p, actname):
                pt0 = psp.tile([P, 512], FP, tag="bank", name=f"p{p}hb0")
                pt1 = psp.tile([P, 512], FP, tag="bank", name=f"p{p}hb1")
                for b in range(2):
                    rt = rp.tile([P, 4 * M], BF, tag="r")
                    nc.sync.dma_start(rt[:], rh_d[p][:, ts(b, 4 * M)])
                    for jj in range(4):
                        j = 4 * b + jj
                        lhsT = ats["h"][j][:, ts(p, P)]
                        nc.tensor.matmul(pt0[:], lhsT,
                                         rt[:, jj * M: jj * M + 512],
                                         start=(j == 0), stop=(j == KT - 1))
                        nc.tensor.matmul(pt1[:], lhsT,
                                         rt[:, jj * M + 512: (jj + 1) * M],
                                         start=(j == 0), stop=(j == KT - 1))
                gt = gates[p]
                af = getattr(AF, actname)
                quarters = 2 if p == 3 else 1  # fine-grain the o-gate tail
                for bb in range(2):
                    pt = (pt0, pt1)[bb]
                    for q in range(quarters):
                        w = 512 // quarters
                        lo = bb * 512 + q * w
                        nc.vector.tensor_add(pt[:, ts(q, w)], pt[:, ts(q, w)],
                                             pxs[p][:, lo: lo + w])
                        nc.scalar.activation(gt[:, lo: lo + w],
                                             pt[:, ts(q, w)], af)

            # x-passes of mm2 interleaved with mm1-h waves: the PE stays busy
            # while R streams, and consumes R just behind the arrival front
            mm1h_wave(0)
            pair_x(0)
            mm1h_wave(1)
            pair_x(1)
            mm1h_wave(2)
            pair_x(2)
            mm1h_wave(3)
            pair_x(3)

            pair_h(0, PAIRS[0][2])
            pair_h(1, PAIRS[1][2])
            pair_h(2, PAIRS[2][2])
            gi, gf, gg = gates[0], gates[1], gates[2]

            # c_new chain overlaps the o-gate matmuls; stores go on the
            # Activation HWDGE ring to keep the SP ring free for R streaming
            fc = ew.tile([P, M], FP, tag="fc")
            ig = ew.tile([P, M], FP, tag="ig")
            cn = ew.tile([P, M], BF, tag="cn")
            tch_t = ew.tile([P, M], BF, tag="tch")
            for hf in range(2):
                sl = ts(hf, 512)
                nc.vector.tensor_mul(fc[:, sl], gf[:, sl], cs[:, sl])
                nc.vector.tensor_mul(ig[:, sl], gi[:, sl], gg[:, sl])
                nc.vector.tensor_add(cn[:, sl], fc[:, sl], ig[:, sl])
                nc.scalar.dma_start(cn_d[:, sl], cn[:, sl])
                nc.scalar.activation(tch_t[:, sl], cn[:, sl], AF.Tanh)

            pair_h(3, PAIRS[3][2])  # o
            go = gates[3]
            hn = ew.tile([P, M], BF, tag="hn")
            for qf in range(4):  # quarters: shortens the post-last-matmul tail
                sl = ts(qf, 256)
                nc.vector.tensor_mul(hn[:, sl], go[:, sl], tch_t[:, sl])
                nc.scalar.dma_start(hn_d[:, sl], hn[:, sl])

    nc.compile()
    return nc


def _get_program():
    if "nc" not in _cache:
        _cache["nc"] = _build_program()
    return _cache["nc"]


def _pack(a):
    # [R*128, C] -> [128, R*C]; out[p, k*C+c] = a[k*128+p, c]
    r = a.shape[0] // P
    return np.ascontiguousarray(
        a.reshape(r, P, a.shape[1]).transpose(1, 0, 2).reshape(P, r * a.shape[1]))


def _prep_in_maps(inputs):
    import ml_dtypes
    BF = ml_dtypes.bfloat16
    bf = lambda a: np.asarray(a, dtype=np.float32).astype(BF)
    xp = _pack(bf(inputs["x"]))
    hp = _pack(bf(inputs["h"]))
    c = np.asarray(inputs["c"], dtype=np.float32)
    LTx = [bf(np.asarray(inputs[f"L_{xg}"]).T) for xg, _, _ in PAIRS]
    LTh = [bf(np.asarray(inputs[f"L_{hg}"]).T) for _, hg, _ in PAIRS]
    Rxp = [_pack(bf(inputs[f"R_{xg}"])) for xg, _, _ in PAIRS]
    Rhp = [_pack(bf(inputs[f"R_{hg}"])) for _, hg, _ in PAIRS]
    bsum = [(np.asarray(inputs[f"b_{xg}"], dtype=np.float32)
             + np.asarray(inputs[f"b_{hg}"], dtype=np.float32)).astype(BF)
            for xg, hg, _ in PAIRS]

    in_maps = []
    for k in range(NC):
        sl = slice(P * k, P * (k + 1))
        im = {
            "xp": xp, "hp": hp,
            "ltxp": _pack(np.ascontiguousarray(
                np.concatenate([lt[:, sl] for lt in LTx], axis=1))),
            "lthp": _pack(np.ascontiguousarray(
                np.concatenate([lt[:, sl] for lt in LTh], axis=1))),
            "bsp": _pack(np.ascontiguousarray(
                np.concatenate([b[sl] for b in bsum], axis=0))),
            "cprev": np.ascontiguousarray(c[sl].astype(BF)),
        }
        for p in range(4):
            im[f"rxp{p}"] = Rxp[p]
            im[f"rhp{p}"] = Rhp[p]
        in_maps.append(im)
    return in_maps


def kernel(**inputs):
    from concourse.bass_utils import run_bass_kernel_spmd

    nc = _get_program()
    in_maps = _prep_in_maps(inputs)
    res = run_bass_kernel_spmd(nc, in_maps, core_ids=list(range(NC)))
    h_new = np.concatenate(
        [np.asarray(res.results[k]["h_new"], dtype=np.float32) for k in range(NC)],
        axis=0)
    c_new = np.concatenate(
        [np.asarray(res.results[k]["c_new"], dtype=np.float32) for k in range(NC)],
        axis=0)
    return (h_new, c_new)


# revision 12
# speedup vs baseline: 1.0048x; 1.0025x over previous
"""KroneckerLSTM trn2 kernel.

Computes, for 8 gate-klins (L @ t @ R + b, t in {x,h}):
    i = sigmoid(klin_ii(x) + klin_hi(h)); f = sigmoid(...); g = tanh(...); o = sigmoid(...)
    c_new = f*c + i*g ; h_new = o*tanh(c_new)
Returns (h_new, c_new), each [1024,1024] f32.

Sharding: output rows split across 8 cores (128 rows each) -> zero collectives.
Per core, for each gate g:  B_g[rows,:] = (L_g[rows,:] @ t) @ R_g
  mm1 computes A^T directly (lhsT = t tiles (natural), rhs = host-pretransposed
  L^T column-slices, stacked 4 gates wide so N=512), so mm1's PSUM output is the
  lhsT for mm2 (rhs = R_g in natural layout).  The x-klin and h-klin of each
  gate pair accumulate into the same PSUM bank; bias is added in-place in PSUM.

All DMA'd tensors are bf16 (host-side cast): per-core HBM traffic is ~24MB.
DMA-instruction issue is descriptor-rate-limited (~5ns per partition line), so
all big tensors are host-packed into [128, K*freedim] layouts whose partition
lines are 8KB-contiguous in DRAM; each DMA instruction then moves 0.5-2MB with
only 128 descriptor lines.  Input loads stream on the SP HWDGE ring in
consumption order; output stores use the Activation ring so they never block
input streaming.
"""

import sys

import numpy as np

if "/opt/trn_rl_repo" not in sys.path:
    sys.path.insert(0, "/opt/trn_rl_repo")

N = 1024
M = 1024
P = 128
NC = 8
KT = N // P  # 8 k-tiles of 128
# gate pairs in order i, f, g, o: (x-gate, h-gate, activation)
PAIRS = [("ii", "hi", "Sigmoid"), ("if", "hf", "Sigmoid"),
         ("ig", "hg", "Tanh"), ("io", "ho", "Sigmoid")]

_cache = {}


def _build_program():
    import concourse.bass as bass
    import concourse.mybir as mybir
    import concourse.tile as tile
    from concourse import bacc
    from concourse.bass import ts

    FP = mybir.dt.float32
    BF = mybir.dt.bfloat16
    AF = mybir.ActivationFunctionType

    nc = bacc.Bacc("TRN2", target_bir_lowering=False, debug=False,
                   enable_asserts=False, num_devices=NC)

    # packed layouts: [128, K*freedim], partition p / chunk k = source row k*128+p
    xp_d = nc.dram_tensor("xp", [P, KT * M], BF, kind="ExternalInput").ap()
    hp_d = nc.dram_tensor("hp", [P, KT * M], BF, kind="ExternalInput").ap()
    ltx_d = nc.dram_tensor("ltxp", [P, KT * 512], BF, kind="ExternalInput").ap()
    lth_d = nc.dram_tensor("lthp", [P, KT * 512], BF, kind="ExternalInput").ap()
    rx_d = [nc.dram_tensor(f"rxp{p}", [P, KT * M], BF, kind="ExternalInput").ap()
            for p in range(4)]
    rh_d = [nc.dram_tensor(f"rhp{p}", [P, KT * M], BF, kind="ExternalInput").ap()
            for p in range(4)]
    bs_d = nc.dram_tensor("bsp", [P, 4 * M], BF, kind="ExternalInput").ap()
    c_d = nc.dram_tensor("cprev", [P, M], BF, kind="ExternalInput").ap()
    hn_d = nc.dram_tensor("h_new", [P, M], BF, kind="ExternalOutput").ap()
    cn_d = nc.dram_tensor("c_new", [P, M], BF, kind="ExternalOutput").ap()

    with tile.TileContext(nc) as tc:
        from contextlib import ExitStack
        with ExitStack() as ctx:
            big = ctx.enter_context(tc.tile_pool(name="big", bufs=1))
            atp = ctx.enter_context(tc.tile_pool(name="at", bufs=1))
            rp = ctx.enter_context(tc.tile_pool(name="rstream", bufs=6))
            psp = ctx.enter_context(tc.tile_pool(name="ps", bufs=8, space="PSUM"))
            gp = ctx.enter_context(tc.tile_pool(name="gates", bufs=1))
            ew = ctx.enter_context(tc.tile_pool(name="ew", bufs=1))
            wp = ctx.enter_context(tc.tile_pool(name="warm", bufs=1))

            # small PE warm-up burst overlapping the DMA prologue
            wa = wp.tile([P, P], BF, tag="wa")
            wb = wp.tile([P, 512], BF, tag="wb")
            nc.vector.memset(wa[:], 0.0)
            nc.vector.memset(wb[:], 0.0)
            wps = psp.tile([P, 512], FP, tag="bank", name="warm_ps")
            for w in range(12):
                nc.tensor.matmul(wps[:], wa[:], wb[:], start=True, stop=True,
                                 skip_group_check=True)

            # ---- all input loads, issued upfront in consumption order (SP
            # ring, FIFO).  x/h stream per k-tile so mm1 starts ~1us after the
            # first bytes land; the very first k-tile is split finer to beat
            # the per-DMA completion latency.
            ltxt = big.tile([P, KT * 512], BF, tag="ltx")
            nc.sync.dma_start(ltxt[:, 0:512], ltx_d[:, 0:512])
            xcs = [big.tile([P, M], BF, tag=f"xc{b}", name=f"xc{b}")
                   for b in range(KT)]
            nc.sync.dma_start(xcs[0][:, 0:512], xp_d[:, 0:512])
            nc.sync.dma_start(xcs[0][:, 512:M], xp_d[:, 512:M])
            nc.sync.dma_start(ltxt[:, 512:KT * 512], ltx_d[:, 512:KT * 512])
            for b in range(1, KT):
                nc.sync.dma_start(xcs[b][:], xp_d[:, ts(b, M)])
            ltht = big.tile([P, KT * 512], BF, tag="lth")
            nc.sync.dma_start(ltht[:], lth_d[:])
            hcs = [big.tile([P, M], BF, tag=f"hc{b}", name=f"hc{b}")
                   for b in range(KT)]
            for b in range(KT):
                nc.sync.dma_start(hcs[b][:], hp_d[:, ts(b, M)])
            bst = big.tile([P, 4 * M], BF, tag="bs")
            nc.sync.dma_start(bst[:], bs_d[:])
            cs = ew.tile([P, M], BF, tag="cs")
            nc.sync.dma_start(cs[:], c_d[:])

            # mm1-x: at_x[j][mloc, 4*128] = sum_k x[k, j*128+mloc] * LTx[k, col]
            ats = {"x": [None] * KT, "h": [None] * KT}
            pts = [psp.tile([P, 4 * P], FP, tag="bank", name=f"pt_x_{j}")
                   for j in range(KT)]
            for kc in range(KT):
                tt = xcs[kc]
                lts = ltxt[:, ts(kc, 512)]
                for j in range(KT):
                    nc.tensor.matmul(pts[j][:], tt[:, ts(j, P)], lts,
                                     start=(kc == 0), stop=(kc == KT - 1))
            for j in range(KT):
                at = atp.tile([P, 4 * P], BF, tag=f"atx{j}", name=f"atx{j}")
                nc.vector.tensor_copy(at[:], pts[j][:])
                ats["x"][j] = at

            # SBUF fp32 partials for the mm2 x-passes (bias folded in)
            pxp = ctx.enter_context(tc.tile_pool(name="px", bufs=1))
            pxs = [pxp.tile([P, M], FP, tag=f"px{p}", name=f"px{p}")
                   for p in range(4)]
            gates = [gp.tile([P, M], BF, tag=f"g{p}", name=f"g{p}")
                     for p in range(4)]

            def mm1h_col(j):
                # one mm1-h output column j (8 MMs, depends only on h/lth):
                # padding work in front of each R-chunk wait so the PE never
                # idles long enough to trip the HAM power throttle
                pw = psp.tile([P, 4 * P], FP, tag="bank", name=f"pt_h_{j}")
                for kc in range(KT):
                    nc.tensor.matmul(pw[:], hcs[kc][:, ts(j, P)],
                                     ltht[:, ts(kc, 512)],
                                     start=(kc == 0), stop=(kc == KT - 1))
                at = atp.tile([P, 4 * P], BF, tag=f"ath{j}", name=f"ath{j}")
                nc.vector.tensor_copy(at[:], pw[:])
                ats["h"][j] = at

            px_banks = {}

            def pair_x_chunk(p, b):
                if b == 0:
                    px_banks[p] = (
                        psp.tile([P, 512], FP, tag="bank", name=f"p{p}xb0"),
                        psp.tile([P, 512], FP, tag="bank", name=f"p{p}xb1"))
                pt0, pt1 = px_banks[p]
                rt = rp.tile([P, 4 * M], BF, tag="r")
                nc.sync.dma_start(rt[:], rx_d[p][:, ts(b, 4 * M)])
                for jj in range(4):
                    j = 4 * b + jj
                    lhsT = ats["x"][j][:, ts(p, P)]
                    nc.tensor.matmul(pt0[:], lhsT,
                                     rt[:, jj * M: jj * M + 512],
                                     start=(j == 0), stop=(j == KT - 1))
                    nc.tensor.matmul(pt1[:], lhsT,
                                     rt[:, jj * M + 512: (jj + 1) * M],
                                     start=(j == 0), stop=(j == KT - 1))
                if b == 1:
                    # copy out of PSUM, pair bias folded in (frees the banks)
                    nc.vector.tensor_add(pxs[p][:, 0:512], pt0[:],
                                         bst[:, p * M: p * M + 512])
                    nc.vector.tensor_add(pxs[p][:, 512:M], pt1[:],
                                         bst[:, p * M + 512: (p + 1) * M])

            def pair_h(p, actname):
                pt0 = psp.tile([P, 512], FP, tag="bank", name=f"p{p}hb0")
                pt1 = psp.tile([P, 512], FP, tag="bank", name=f"p{p}hb1")
                for b in range(2):
                    rt = rp.tile([P, 4 * M], BF, tag="r")
                    nc.sync.dma_start(rt[:], rh_d[p][:, ts(b, 4 * M)])
                    for jj in range(4):
                        j = 4 * b + jj
                        lhsT = ats["h"][j][:, ts(p, P)]
                        nc.tensor.matmul(pt0[:], lhsT,
                                         rt[:, jj * M: jj * M + 512],
                                         start=(j == 0), stop=(j == KT - 1))
                        nc.tensor.matmul(pt1[:], lhsT,
                                         rt[:, jj * M + 512: (jj + 1) * M],
                                         start=(j == 0), stop=(j == KT - 1))
                gt = gates[p]
                af = getattr(AF, actname)
                quarters = 2 if p == 3 else 1  # fine-grain the o-gate tail
                for bb in range(2):
                    pt = (pt0, pt1)[bb]
                    for q in range(quarters):
                        w = 512 // quarters
                        lo = bb * 512 + q * w
                        nc.vector.tensor_add(pt[:, ts(q, w)], pt[:, ts(q, w)],
                                             pxs[p][:, lo: lo + w])
                        nc.scalar.activation(gt[:, lo: lo + w],
                                             pt[:, ts(q, w)], af)

            # x-passes of mm2 interleaved with mm1-h columns: the PE stays
            # busy while R streams, consuming R just behind the arrival front
            k = 0
            for p in range(4):
                for b in range(2):
                    mm1h_col(k)
                    k += 1
                    pair_x_chunk(p, b)

            pair_h(0, PAIRS[0][2])
            pair_h(1, PAIRS[1][2])
            pair_h(2, PAIRS[2][2])
            gi, gf, gg = gates[0], gates[1], gates[2]

            # c_new chain overlaps the o-gate matmuls; stores go on the
            # Activation HWDGE ring to keep the SP ring free for R streaming
            fc = ew.tile([P, M], FP, tag="fc")
            ig = ew.tile([P, M], FP, tag="ig")
            cn = ew.tile([P, M], BF, tag="cn")
            tch_t = ew.tile([P, M], BF, tag="tch")
            for hf in range(2):
                sl = ts(hf, 512)
                nc.vector.tensor_mul(fc[:, sl], gf[:, sl], cs[:, sl])
                nc.vector.tensor_mul(ig[:, sl], gi[:, sl], gg[:, sl])
                nc.vector.tensor_add(cn[:, sl], fc[:, sl], ig[:, sl])
                nc.scalar.dma_start(cn_d[:, sl], cn[:, sl])
                nc.scalar.activation(tch_t[:, sl], cn[:, sl], AF.Tanh)

            pair_h(3, PAIRS[3][2])  # o
            go = gates[3]
            hn = ew.tile([P, M], BF, tag="hn")
            for qf in range(4):  # quarters: shortens the post-last-matmul tail
                sl = ts(qf, 256)
                nc.vector.tensor_mul(hn[:, sl], go[:, sl], tch_t[:, sl])
                nc.scalar.dma_start(hn_d[:, sl], hn[:, sl])

    nc.compile()
    return nc


def _get_program():
    if "nc" not in _cache:
        _cache["nc"] = _build_program()
    return _cache["nc"]


def _pack(a):
    # [R*128, C] -> [128, R*C]; out[p, k*C+c] = a[k*128+p, c]
    r = a.shape[0] // P
    return np.ascontiguousarray(
        a.reshape(r, P, a.shape[1]).transpose(1, 0, 2).reshape(P, r * a.shape[1]))


def _prep_in_maps(inputs):
    import ml_dtypes
    BF = ml_dtypes.bfloat16
    bf = lambda a: np.asarray(a, dtype=np.float32).astype(BF)
    xp = _pack(bf(inputs["x"]))
    hp = _pack(bf(inputs["h"]))
    c = np.asarray(inputs["c"], dtype=np.float32)
    LTx = [bf(np.asarray(inputs[f"L_{xg}"]).T) for xg, _, _ in PAIRS]
    LTh = [bf(np.asarray(inputs[f"L_{hg}"]).T) for _, hg, _ in PAIRS]
    Rxp = [_pack(bf(inputs[f"R_{xg}"])) for xg, _, _ in PAIRS]
    Rhp = [_pack(bf(inputs[f"R_{hg}"])) for _, hg, _ in PAIRS]
    bsum = [(np.asarray(inputs[f"b_{xg}"], dtype=np.float32)
             + np.asarray(inputs[f"b_{hg}"], dtype=np.float32)).astype(BF)
            for xg, hg, _ in PAIRS]

    in_maps = []
    for k in range(NC):
        sl = slice(P * k, P * (k + 1))
        im = {
            "xp": xp, "hp": hp,
            "ltxp": _pack(np.ascontiguousarray(
                np.concatenate([lt[:, sl] for lt in LTx], axis=1))),
            "lthp": _pack(np.ascontiguousarray(
                np.concatenate([lt[:, sl] for lt in LTh], axis=1))),
            "bsp": _pack(np.ascontiguousarray(
                np.concatenate([b[sl] for b in bsum], axis=0))),
            "cprev": np.ascontiguousarray(c[sl].astype(BF)),
        }
        for p in range(4):
            im[f"rxp{p}"] = Rxp[p]
            im[f"rhp{p}"] = Rhp[p]
        in_maps.append(im)
    return in_maps


def kernel(**inputs):
    from concourse.bass_utils import run_bass_kernel_spmd

    nc = _get_program()
    in_maps = _prep_in_maps(inputs)
    res = run_bass_kernel_spmd(nc, in_maps, core_ids=list(range(NC)))
    h_new = np.concatenate(
        [np.asarray(res.results[k]["h_new"], dtype=np.float32) for k in range(NC)],
        axis=0)
    c_new = np.concatenate(
        [np.asarray(res.results[k]["c_new"], dtype=np.float32) for k in range(NC)],
        axis=0)
    return (h_new, c_new)
